# revision 12
# baseline (speedup 1.0000x reference)
"""Chunkwise causal attention (B=2, S=4096, H=16, D=64, CHUNK=128) on 8 TRN2 NeuronCores.

Sharding: head-parallel tensor parallelism. Core c owns heads (2c, 2c+1) for both
batches: it computes the qkv projection for its heads (w_qkv column slice), full
causal attention for its 4 (batch, head) units, and a partial out-projection
(w_out row slice). Host sums the 8 partial outputs.

Device kernel layout notes (v2):
 - x is passed host-transposed as xT [1024, 8192] bf16 so the qkv contraction dim
   (hidden) lands on SBUF partitions without any device-side transpose.
 - Q^T, K^T are kept head-major [128 = 2 heads x 64, S]; V is kept key-major
   [keys, 2, 65] with a ones column so the P@V matmul also produces the softmax
   denominator (row 64 of the PV psum).
 - Scores are computed transposed (scores^T [keys, queries]); both heads of one
   key chunk share one [128, 1024] psum tile (h0 cols 0:512, h1 cols 512:1024)
   so a single ACT exp call covers the chunk and the sq pool double-buffers
   across chunks within 4 psum banks.
 - Projections accumulate into the same sq-pool banks (Q|K in one tile, V in
   another), so the mm pool is exclusively for the out-projection: no psum
   contention between group g+1's projections and group g-1's out-projection.
 - Softmax skips max-subtraction (scores ~ N(0,1): exp never overflows in f32);
   causal masking multiplies the exp'd diagonal blocks by a precomputed 0/1 mask
   packed per-jg as [tri(W) | zeros(qoff) | tri(W)].
 - 1/denominator via nc.vector.reciprocal emitted one group early so its
   ~6.5us single-partition cost overlaps the next group's projections instead
   of stalling the out-projection chain; ones-broadcast matmuls spread it.
 - xT group slices are prefetched two groups ahead on the sync DGE queue.
"""

import sys

if "/opt/trn_rl_repo" not in sys.path:
    sys.path.insert(0, "/opt/trn_rl_repo")

import numpy as np
import ml_dtypes

B = 2
S = 4096
HID = 1024
NHEAD = 16
D = 64
CH = 128  # key chunk (= reference CHUNK)
G = 512  # query group (4 chunks)
NGB = S // G  # 8 query groups per batch
KK = HID // 128  # 8 contraction chunks for the projections
NKC = S // CH  # 32 key chunks per batch
TT = B * S  # 8192 tokens across batches

_CACHE = {}


def _build_nc(reps=1):
    import concourse.mybir as mybir
    import concourse.tile as tile
    from concourse import bacc
    from contextlib import ExitStack

    bf16 = mybir.dt.bfloat16
    f32 = mybir.dt.float32
    Exp = mybir.ActivationFunctionType.Exp
    mult = mybir.AluOpType.mult

    MASKW = 4 * 1024 - (0 + 128 + 256 + 384)  # 3328 packed mask cols

    nc = bacc.Bacc("TRN2", target_bir_lowering=False, debug=False)
    xT_d = nc.dram_tensor("xT", [HID, TT], bf16, kind="ExternalInput")
    wq_d = nc.dram_tensor("wq", [HID, 128], bf16, kind="ExternalInput")
    wk_d = nc.dram_tensor("wk", [HID, 128], bf16, kind="ExternalInput")
    wv_d = nc.dram_tensor("wv", [HID, 128], bf16, kind="ExternalInput")
    wo_d = nc.dram_tensor("wo", [128, HID], bf16, kind="ExternalInput")
    mask_d = nc.dram_tensor("mask", [128, MASKW], bf16, kind="ExternalInput")
    out_d = nc.dram_tensor("out", [TT, HID], f32, kind="ExternalOutput")

    xT_r = xT_d.rearrange("(kk p) t -> p kk t", p=128)
    wq_r = wq_d.rearrange("(kk p) c -> p kk c", p=128)
    wk_r = wk_d.rearrange("(kk p) c -> p kk c", p=128)
    wv_r = wv_d.rearrange("(kk p) c -> p kk c", p=128)

    # packed mask offsets per jg (widths 1024, 896, 768, 640)
    MOFF = [0, 1024, 1920, 2688]

    with tile.TileContext(nc) as tc, ExitStack() as ctx:
        consts = ctx.enter_context(tc.tile_pool(name="consts", bufs=1))
        qkv_pool = ctx.enter_context(tc.tile_pool(name="qkv", bufs=1))
        xt_pool = ctx.enter_context(tc.tile_pool(name="xt", bufs=4))
        exp_pool = ctx.enter_context(tc.tile_pool(name="exp", bufs=6))
        attn_pool = ctx.enter_context(tc.tile_pool(name="attn", bufs=2))
        norm_pool = ctx.enter_context(tc.tile_pool(name="norm", bufs=2))
        osb_pool = ctx.enter_context(tc.tile_pool(name="osb", bufs=3))
        ps_sq = ctx.enter_context(tc.tile_pool(name="pssq", bufs=2, space="PSUM"))
        ps_pv = ctx.enter_context(tc.tile_pool(name="pspv", bufs=2, space="PSUM"))
        ps_mm = ctx.enter_context(tc.tile_pool(name="psmm", bufs=2, space="PSUM"))

        wq_sb = consts.tile([128, KK, 128], bf16, tag="wq")
        wk_sb = consts.tile([128, KK, 128], bf16, tag="wk")
        wv_sb = consts.tile([128, KK, 128], bf16, tag="wv")
        wo_sb = consts.tile([128, HID], bf16, tag="wo")
        mask_sb = consts.tile([128, MASKW], bf16, tag="mask")
        ones_sb = consts.tile([1, 64], bf16, tag="ones")
        nc.vector.memset(ones_sb[:], 1.0)

        bodies = [(gg, bb) for gg in range(NGB) for bb in range(B)]

        for _rep in range(reps):
            QTb, KTb, Vb = [], [], []
            for b in range(B):
                QTb.append(qkv_pool.tile([128, S], bf16, tag=f"QT{b}", name=f"QT{b}"))
                KTb.append(qkv_pool.tile([128, S], bf16, tag=f"KT{b}", name=f"KT{b}"))
                Vb.append(qkv_pool.tile([128, NKC, 2, 65], bf16, tag=f"V{b}", name=f"V{b}"))
                nc.gpsimd.memset(Vb[b][:, :, :, 64:65], 1.0)

            xts = {}
            if _rep == 0:
                # interleave const loads with the first two xt slices so the
                # first Q matmul only waits on wq + xt(0).
                nc.sync.dma_start(wq_sb[:], wq_r)
            for idx in range(2):
                g, b = bodies[idx]
                t0 = b * S + g * G
                xts[idx] = xt_pool.tile([128, KK, G], bf16, tag="xt", name="xt")
                nc.sync.dma_start(xts[idx][:], xT_r[:, :, t0 : t0 + G])
            if _rep == 0:
                nc.sync.dma_start(wk_sb[:], wk_r)
                nc.scalar.dma_start(wv_sb[:], wv_r)
                nc.scalar.dma_start(mask_sb[:], mask_d[:])
                nc.scalar.dma_start(wo_sb[:], wo_d[:])

            # pending finish work from the previous body:
            #   (pvs, at, rec, bcast, t0)
            pending = None

            for idx, (g, b) in enumerate(bodies):
                QT, KT, V = QTb[b], KTb[b], Vb[b]
                t0 = b * S + g * G

                # ---- prefetch xt two bodies ahead ----
                if idx + 2 < len(bodies):
                    g2, b2 = bodies[idx + 2]
                    t02 = b2 * S + g2 * G
                    xts[idx + 2] = xt_pool.tile([128, KK, G], bf16, tag="xt", name="xt")
                    nc.sync.dma_start(xts[idx + 2][:], xT_r[:, :, t02 : t02 + G])

                xt = xts.pop(idx)

                # ---- phase 1: qkv projection for this token group ----
                # Q and K accumulate into the two banks of one sq-pool tile.
                qk_ps = ps_sq.tile([128, 2 * G], f32, tag="sq", name="qk_ps")
                for col, w_sb in ((0, wq_sb), (G, wk_sb)):
                    for kk in range(KK):
                        nc.tensor.matmul(
                            qk_ps[:, col : col + G],
                            w_sb[:, kk, :],
                            xt[:, kk, :],
                            start=(kk == 0),
                            stop=(kk == KK - 1),
                        )
                nc.vector.tensor_copy(QT[:, g * G : (g + 1) * G], qk_ps[:, 0:G])
                nc.vector.tensor_copy(KT[:, g * G : (g + 1) * G], qk_ps[:, G : 2 * G])
                v_ps = ps_sq.tile([128, 2 * G], f32, tag="sq", name="v_ps")
                for tch in range(G // CH):
                    # alternate psum banks so each evac overlaps the next
                    # tch's matmuls instead of colliding on one bank
                    vcol = (tch % 2) * G + (tch // 2) * CH
                    for kk in range(KK):
                        nc.tensor.matmul(
                            v_ps[:, vcol : vcol + CH],
                            xt[:, kk, tch * CH : (tch + 1) * CH],
                            wv_sb[:, kk, :],
                            start=(kk == 0),
                            stop=(kk == KK - 1),
                        )
                    kc = g * 4 + tch
                    nc.vector.tensor_copy(
                        V[:, kc, :, 0:64],
                        v_ps[:, vcol : vcol + CH].rearrange("p (h d) -> p h d", h=2),
                    )

                # ---- phase 2: attention for query group g (keys 0..4g+3) ----
                nkc = 4 * g + 4  # causal key chunks for this group
                pv = [
                    ps_pv.tile([65, G], f32, tag="pv", name=f"pv{h}") for h in range(2)
                ]
                recip_pieces = []
                if pending is not None:
                    pvs_p, at_p, recb_p, _ = pending
                    NP = 4
                    W = 2 * G // NP
                    recip_pieces = [(recb_p[:, i * W : (i + 1) * W],
                                     pvs_p[64:65, i * W : (i + 1) * W])
                                    for i in range(NP)]
                for kc in range(nkc):
                    if recip_pieces and kc % max(nkc // 4, 1) == 0:
                        dst, srcp = recip_pieces.pop(0)
                        with nc.allow_low_precision(reason="softmax denom recip"):
                            nc.vector.reciprocal(dst, srcp)
                    jg = kc - (nkc - 4)  # >=0 on the diagonal band
                    qoff = max(jg, 0) * CH
                    sq = ps_sq.tile([128, 2 * G], f32, tag="sq", name="sq")
                    # h0 trimmed to [qoff:512]; h1 untrimmed [512:1024] so the
                    # exp region [qoff:1024] is fully written (no stale reads).
                    nc.tensor.matmul(
                        sq[:, qoff:G],
                        KT[0:64, kc * CH : (kc + 1) * CH],
                        QT[0:64, g * G + qoff : (g + 1) * G],
                    )
                    nc.tensor.matmul(
                        sq[:, G : 2 * G],
                        KT[64:128, kc * CH : (kc + 1) * CH],
                        QT[64:128, g * G : (g + 1) * G],
                    )
                    ex = exp_pool.tile([128, 2 * G], bf16, tag="exp", name="ex")
                    # exp(scores / sqrt(D)); scale folded into the ACT affine
                    nc.scalar.activation(
                        ex[:, qoff : 2 * G], sq[:, qoff : 2 * G], Exp, scale=0.125
                    )
                    if jg >= 0:
                        moff = MOFF[jg]
                        mw = 2 * G - qoff
                        nc.vector.tensor_tensor(
                            ex[:, qoff : 2 * G],
                            ex[:, qoff : 2 * G],
                            mask_sb[:, moff : moff + mw],
                            op=mult,
                        )
                    nc.tensor.matmul(
                        pv[0][:, qoff:G],
                        V[:, kc, 0, :],
                        ex[:, qoff:G],
                        start=(kc == 0),
                        stop=(kc == nkc - 1),
                    )
                    nc.tensor.matmul(
                        pv[1][:, qoff:G],
                        V[:, kc, 1, :],
                        ex[:, G + qoff : 2 * G],
                        start=(kc == 0),
                        stop=(kc == nkc - 1),
                    )

                for dst, srcp in recip_pieces:
                    with nc.allow_low_precision(reason="softmax denom recip"):
                        nc.vector.reciprocal(dst, srcp)

                # ---- at-multiplies for the previous body (rec ready) ----
                if pending is not None:
                    pvs_p, at_p, recb_p, t0_p = pending
                    bcp = ps_mm.tile([128, G], f32, tag="mm", name="bcp")
                    nc.tensor.matmul(bcp[0:64, :], ones_sb[:], recb_p[0:1, 0:G])
                    nc.tensor.matmul(
                        bcp[64:128, :], ones_sb[:], recb_p[0:1, G : 2 * G],
                        tile_position=(0, 64),
                    )
                    nc.vector.tensor_tensor(
                        at_p[0:64, :], pvs_p[0:64, 0:G], bcp[0:64, :], op=mult
                    )
                    nc.vector.tensor_tensor(
                        at_p[64:128, :], pvs_p[0:64, G : 2 * G], bcp[64:128, :],
                        op=mult,
                    )

                # ---- evacuate pv psum; queue this body's finish work ----
                pvs = norm_pool.tile([65, 2 * G], f32, tag="pvs")
                nc.vector.tensor_copy(pvs[:, 0:G], pv[0][:])
                nc.vector.tensor_copy(pvs[:, G : 2 * G], pv[1][:])

                # ---- out-projection for the previous body ----
                if pending is not None:
                    _, at_p, _, t0_p = pending
                    outproj(nc, ps_mm, osb_pool, out_d, wo_sb, at_p, t0_p)

                recb = norm_pool.tile([1, 2 * G], bf16, tag="recb")
                at = attn_pool.tile([128, G], bf16, tag="attnT")
                pending = (pvs, at, recb, t0)

            # ---- drain the last body's finish chain ----
            pvs_p, at_p, recb_p, t0_p = pending
            with nc.allow_low_precision(reason="softmax denom recip in bf16"):
                nc.vector.reciprocal(recb_p[:], pvs_p[64:65, :])
            bcp = ps_mm.tile([128, G], f32, tag="mm", name="bcp")
            nc.tensor.matmul(bcp[0:64, :], ones_sb[:], recb_p[0:1, 0:G])
            nc.tensor.matmul(
                bcp[64:128, :], ones_sb[:], recb_p[0:1, G : 2 * G],
                tile_position=(0, 64),
            )
            nc.vector.tensor_tensor(
                at_p[0:64, :], pvs_p[0:64, 0:G], bcp[0:64, :], op=mult
            )
            nc.vector.tensor_tensor(
                at_p[64:128, :], pvs_p[0:64, G : 2 * G], bcp[64:128, :], op=mult
            )
            outproj(nc, ps_mm, osb_pool, out_d, wo_sb, at_p, t0_p)
    nc.compile()
    return nc


def outproj(nc, ps_mm, osb_pool, out_d, wo_sb, at, t0):
    import concourse.mybir as mybir

    f32 = mybir.dt.float32
    for tch in range(G // CH):
        ob = osb_pool.tile([128, HID], f32, tag="ob")
        for nn in range(2):
            pso = ps_mm.tile([128, G], f32, tag="mm")
            nc.tensor.matmul(
                pso[:],
                at[:, tch * CH : (tch + 1) * CH],
                wo_sb[:, nn * G : (nn + 1) * G],
            )
            nc.vector.tensor_copy(ob[:, nn * G : (nn + 1) * G], pso[:])
        nc.sync.dma_start(
            out_d[t0 + tch * CH : t0 + (tch + 1) * CH, :],
            ob[:],
        )


def _causal_mask():
    # Packed per-region triangular mask. Diagonal-band chunk jg keeps queries
    # qq >= k within its W = 512 - 128*jg window; layout per jg is
    # [tri(W) | zeros(qoff) | tri(W)] covering [qoff:1024] of the h0|h1 tile.
    tri = (np.arange(G)[None, :] >= np.arange(128)[:, None]).astype(np.float32)
    parts = []
    for jg in range(4):
        qoff = jg * CH
        w = G - qoff
        parts.append(tri[:, 0:w])
        if qoff:
            parts.append(np.zeros((128, qoff), dtype=np.float32))
        parts.append(tri[:, 0:w])
    return np.concatenate(parts, axis=1).astype(ml_dtypes.bfloat16)


def get_nc(reps=1):
    key = f"nc{reps}"
    if key not in _CACHE:
        _CACHE[key] = _build_nc(reps)
    return _CACHE[key]


def make_in_maps(x, w_qkv, w_out):
    bf16 = ml_dtypes.bfloat16
    xf = np.asarray(x, dtype=np.float32).reshape(TT, HID)
    xT = np.ascontiguousarray(xf.T).astype(bf16)
    wqkv = np.asarray(w_qkv, dtype=np.float32)
    wout = np.asarray(w_out, dtype=np.float32)
    mask = _causal_mask()
    in_maps = []
    for c in range(8):
        c0 = 128 * c
        in_maps.append(
            {
                "xT": xT,
                "wq": np.ascontiguousarray(wqkv[:, c0 : c0 + 128]).astype(bf16),
                "wk": np.ascontiguousarray(wqkv[:, HID + c0 : HID + c0 + 128]).astype(bf16),
                "wv": np.ascontiguousarray(wqkv[:, 2 * HID + c0 : 2 * HID + c0 + 128]).astype(bf16),
                "wo": np.ascontiguousarray(wout[c0 : c0 + 128, :]).astype(bf16),
                "mask": mask,
            }
        )
    return in_maps


def kernel(x, w_qkv, w_out):
    from concourse.bass_utils import run_bass_kernel_spmd

    nc = get_nc()
    in_maps = make_in_maps(x, w_qkv, w_out)
    res = run_bass_kernel_spmd(nc, in_maps, core_ids=list(range(8)))
    acc = np.zeros((TT, HID), dtype=np.float32)
    for r in res.results:
        acc += r["out"]
    return acc.reshape(B, S, HID)


# revision 15
# speedup vs baseline: 1.2063x; 1.2063x over previous
"""Chunkwise causal attention (B=2, S=4096, H=16, D=64, CHUNK=128) on 8 TRN2 NeuronCores.

Sharding: head-parallel tensor parallelism. Core c owns heads (2c, 2c+1) for both
batches: it computes the qkv projection for its heads (w_qkv column slice), full
causal attention for its 4 (batch, head) units, and a partial out-projection
(w_out row slice). Host sums the 8 partial outputs.

Device kernel layout notes (v2):
 - x is passed host-transposed as xT [1024, 8192] bf16 so the qkv contraction dim
   (hidden) lands on SBUF partitions without any device-side transpose.
 - Q^T, K^T are kept head-major [128 = 2 heads x 64, S]; V is kept key-major
   [keys, 2, 65] with a ones column so the P@V matmul also produces the softmax
   denominator (row 64 of the PV psum).
 - Scores are computed transposed (scores^T [keys, queries]); both heads of one
   key chunk share one [128, 1024] psum tile (h0 cols 0:512, h1 cols 512:1024)
   so a single ACT exp call covers the chunk and the sq pool double-buffers
   across chunks within 4 psum banks.
 - Projections accumulate into the same sq-pool banks (Q|K in one tile, V in
   another), so the mm pool is exclusively for the out-projection: no psum
   contention between group g+1's projections and group g-1's out-projection.
 - Softmax skips max-subtraction (scores ~ N(0,1): exp never overflows in f32);
   causal masking multiplies the exp'd diagonal blocks by a precomputed 0/1 mask
   packed per-jg as [tri(W) | zeros(qoff) | tri(W)].
 - 1/denominator via nc.vector.reciprocal emitted one group early so its
   ~6.5us single-partition cost overlaps the next group's projections instead
   of stalling the out-projection chain; ones-broadcast matmuls spread it.
 - xT group slices are prefetched two groups ahead on the sync DGE queue.
"""

import sys

if "/opt/trn_rl_repo" not in sys.path:
    sys.path.insert(0, "/opt/trn_rl_repo")

import numpy as np
import ml_dtypes

B = 2
S = 4096
HID = 1024
NHEAD = 16
D = 64
CH = 128  # key chunk (= reference CHUNK)
G = 512  # query group (4 chunks)
NGB = S // G  # 8 query groups per batch
KK = HID // 128  # 8 contraction chunks for the projections
NKC = S // CH  # 32 key chunks per batch
TT = B * S  # 8192 tokens across batches

_CACHE = {}


def _build_nc(reps=1):
    import concourse.mybir as mybir
    import concourse.tile as tile
    from concourse import bacc
    from contextlib import ExitStack

    bf16 = mybir.dt.bfloat16
    f32 = mybir.dt.float32
    Exp = mybir.ActivationFunctionType.Exp
    mult = mybir.AluOpType.mult

    MASKW = 4 * 1024 - (0 + 128 + 256 + 384)  # 3328 packed mask cols

    nc = bacc.Bacc("TRN2", target_bir_lowering=False, debug=False)
    xT_d = nc.dram_tensor("xT", [HID, TT], bf16, kind="ExternalInput")
    wq_d = nc.dram_tensor("wq", [HID, 128], bf16, kind="ExternalInput")
    wk_d = nc.dram_tensor("wk", [HID, 128], bf16, kind="ExternalInput")
    wv_d = nc.dram_tensor("wv", [HID, 128], bf16, kind="ExternalInput")
    wo_d = nc.dram_tensor("wo", [128, HID], bf16, kind="ExternalInput")
    mask_d = nc.dram_tensor("mask", [128, MASKW], bf16, kind="ExternalInput")
    ident_d = nc.dram_tensor("ident", [128, 128], f32, kind="ExternalInput")
    out_d = nc.dram_tensor("out", [TT, HID], f32, kind="ExternalOutput")

    xT_r = xT_d.rearrange("(kk p) t -> p kk t", p=128)
    wq_r = wq_d.rearrange("(kk p) c -> p kk c", p=128)
    wk_r = wk_d.rearrange("(kk p) c -> p kk c", p=128)
    wv_r = wv_d.rearrange("(kk p) c -> p kk c", p=128)

    # packed mask offsets per jg (widths 1024, 896, 768, 640)
    MOFF = [0, 1024, 1920, 2688]

    with tile.TileContext(nc) as tc, ExitStack() as ctx:
        consts = ctx.enter_context(tc.tile_pool(name="consts", bufs=1))
        qkv_pool = ctx.enter_context(tc.tile_pool(name="qkv", bufs=1))
        xt_pool = ctx.enter_context(tc.tile_pool(name="xt", bufs=4))
        exp_pool = ctx.enter_context(tc.tile_pool(name="exp", bufs=6))
        attn_pool = ctx.enter_context(tc.tile_pool(name="attn", bufs=2))
        norm_pool = ctx.enter_context(tc.tile_pool(name="norm", bufs=2))
        osb_pool = ctx.enter_context(tc.tile_pool(name="osb", bufs=3))
        ps_sq = ctx.enter_context(tc.tile_pool(name="pssq", bufs=2, space="PSUM"))
        ps_pv = ctx.enter_context(tc.tile_pool(name="pspv", bufs=2, space="PSUM"))
        ps_mm = ctx.enter_context(tc.tile_pool(name="psmm", bufs=2, space="PSUM"))

        wq_sb = consts.tile([128, KK, 128], bf16, tag="wq")
        wk_sb = consts.tile([128, KK, 128], bf16, tag="wk")
        wv_sb = consts.tile([128, KK, 128], bf16, tag="wv")
        wo_sb = consts.tile([128, HID], bf16, tag="wo")
        mask_sb = consts.tile([128, MASKW], bf16, tag="mask")
        ones_sb = consts.tile([1, 64], bf16, tag="ones")
        nc.vector.memset(ones_sb[:], 1.0)
        ones1_sb = consts.tile([128, 1], f32, tag="ones1")
        nc.vector.memset(ones1_sb[:], 1.0)
        ident_sb = consts.tile([128, 128], f32, tag="ident")
        nc.scalar.dma_start(ident_sb[:], ident_d[:])

        bodies = [(gg, bb) for gg in range(NGB) for bb in range(B)]

        for _rep in range(reps):
            QTb, KTb, Vb = [], [], []
            for b in range(B):
                QTb.append(qkv_pool.tile([128, S], bf16, tag=f"QT{b}", name=f"QT{b}"))
                KTb.append(qkv_pool.tile([128, S], bf16, tag=f"KT{b}", name=f"KT{b}"))
                Vb.append(qkv_pool.tile([128, NKC, 2, 65], bf16, tag=f"V{b}", name=f"V{b}"))
                nc.gpsimd.memset(Vb[b][:, :, :, 64:65], 1.0)

            xts = {}
            if _rep == 0:
                # interleave const loads with the first two xt slices so the
                # first Q matmul only waits on wq + xt(0).
                nc.sync.dma_start(wq_sb[:], wq_r)
            for idx in range(2):
                g, b = bodies[idx]
                t0 = b * S + g * G
                xts[idx] = xt_pool.tile([128, KK, G], bf16, tag="xt", name="xt")
                nc.sync.dma_start(xts[idx][:], xT_r[:, :, t0 : t0 + G])
            if _rep == 0:
                nc.sync.dma_start(wk_sb[:], wk_r)
                nc.scalar.dma_start(wv_sb[:], wv_r)
                nc.scalar.dma_start(mask_sb[:], mask_d[:])
                nc.scalar.dma_start(wo_sb[:], wo_d[:])

            # pending finish work from the previous body:
            #   (pvs, at, rec, bcast, t0)
            pending = None

            for idx, (g, b) in enumerate(bodies):
                QT, KT, V = QTb[b], KTb[b], Vb[b]
                t0 = b * S + g * G

                # ---- prefetch xt two bodies ahead ----
                if idx + 2 < len(bodies):
                    g2, b2 = bodies[idx + 2]
                    t02 = b2 * S + g2 * G
                    xts[idx + 2] = xt_pool.tile([128, KK, G], bf16, tag="xt", name="xt")
                    nc.sync.dma_start(xts[idx + 2][:], xT_r[:, :, t02 : t02 + G])

                xt = xts.pop(idx)

                # ---- phase 1: qkv projection for this token group ----
                # Q and K accumulate into the two banks of one sq-pool tile.
                qk_ps = ps_sq.tile([128, 2 * G], f32, tag="sq", name="qk_ps")
                for col, w_sb in ((0, wq_sb), (G, wk_sb)):
                    for kk in range(KK):
                        nc.tensor.matmul(
                            qk_ps[:, col : col + G],
                            w_sb[:, kk, :],
                            xt[:, kk, :],
                            start=(kk == 0),
                            stop=(kk == KK - 1),
                        )
                nc.vector.tensor_copy(QT[:, g * G : (g + 1) * G], qk_ps[:, 0:G])
                nc.vector.tensor_copy(KT[:, g * G : (g + 1) * G], qk_ps[:, G : 2 * G])
                v_ps = ps_sq.tile([128, 2 * G], f32, tag="sq", name="v_ps")
                for tch in range(G // CH):
                    # alternate psum banks so each evac overlaps the next
                    # tch's matmuls instead of colliding on one bank
                    vcol = (tch % 2) * G + (tch // 2) * CH
                    for kk in range(KK):
                        nc.tensor.matmul(
                            v_ps[:, vcol : vcol + CH],
                            xt[:, kk, tch * CH : (tch + 1) * CH],
                            wv_sb[:, kk, :],
                            start=(kk == 0),
                            stop=(kk == KK - 1),
                        )
                    kc = g * 4 + tch
                    nc.vector.tensor_copy(
                        V[:, kc, :, 0:64],
                        v_ps[:, vcol : vcol + CH].rearrange("p (h d) -> p h d", h=2),
                    )

                # ---- recip for the previous body on 128 partitions ----
                # denominators [1,1024] -> [128 tokens, 8 blocks] via 8 tiny
                # column-broadcast matmuls, one [128,8] DVE reciprocal, then a
                # PE transpose back to row layout for the bcp broadcasts.
                if pending is not None:
                    pvs_p, at_p, recb_p, _ = pending
                    dT = ps_mm.tile([128, G], f32, tag="mm", name="dT")
                    for c in range(8):
                        nc.tensor.matmul(
                            dT[:, c : c + 1],
                            pvs_p[64:65, c * CH : (c + 1) * CH],
                            ones1_sb[64:65, :],
                        )
                    recT = norm_pool.tile([128, 8], f32, tag="recT")
                    nc.vector.reciprocal(recT[:], dT[:, 0:8])
                    rtp = ps_mm.tile([128, G], f32, tag="mm", name="rtp")
                    nc.tensor.transpose(rtp[0:8, 0:128], recT[:], ident_sb[:])
                    with nc.allow_low_precision(reason="softmax denom recip bf16"):
                        for r in range(8):
                            nc.vector.tensor_copy(
                                recb_p[0:1, r * CH : (r + 1) * CH],
                                rtp[r : r + 1, 0:128],
                            )

                # ---- phase 2: attention for query group g (keys 0..4g+3) ----
                nkc = 4 * g + 4  # causal key chunks for this group
                pv = [
                    ps_pv.tile([65, G], f32, tag="pv", name=f"pv{h}") for h in range(2)
                ]
                for kc in range(nkc):
                    jg = kc - (nkc - 4)  # >=0 on the diagonal band
                    qoff = max(jg, 0) * CH
                    sq = ps_sq.tile([128, 2 * G], f32, tag="sq", name="sq")
                    # h0 trimmed to [qoff:512]; h1 untrimmed [512:1024] so the
                    # exp region [qoff:1024] is fully written (no stale reads).
                    nc.tensor.matmul(
                        sq[:, qoff:G],
                        KT[0:64, kc * CH : (kc + 1) * CH],
                        QT[0:64, g * G + qoff : (g + 1) * G],
                    )
                    nc.tensor.matmul(
                        sq[:, G : 2 * G],
                        KT[64:128, kc * CH : (kc + 1) * CH],
                        QT[64:128, g * G : (g + 1) * G],
                    )
                    ex = exp_pool.tile([128, 2 * G], bf16, tag="exp", name="ex")
                    # exp(scores / sqrt(D)); scale folded into the ACT affine
                    nc.scalar.activation(
                        ex[:, qoff : 2 * G], sq[:, qoff : 2 * G], Exp, scale=0.125
                    )
                    if jg >= 0:
                        moff = MOFF[jg]
                        mw = 2 * G - qoff
                        nc.vector.tensor_tensor(
                            ex[:, qoff : 2 * G],
                            ex[:, qoff : 2 * G],
                            mask_sb[:, moff : moff + mw],
                            op=mult,
                        )
                    nc.tensor.matmul(
                        pv[0][:, qoff:G],
                        V[:, kc, 0, :],
                        ex[:, qoff:G],
                        start=(kc == 0),
                        stop=(kc == nkc - 1),
                    )
                    nc.tensor.matmul(
                        pv[1][:, qoff:G],
                        V[:, kc, 1, :],
                        ex[:, G + qoff : 2 * G],
                        start=(kc == 0),
                        stop=(kc == nkc - 1),
                    )

                # ---- at-multiplies for the previous body (rec ready) ----
                if pending is not None:
                    pvs_p, at_p, recb_p, t0_p = pending
                    bcp = ps_mm.tile([128, G], f32, tag="mm", name="bcp")
                    nc.tensor.matmul(bcp[0:64, :], ones_sb[:], recb_p[0:1, 0:G])
                    nc.tensor.matmul(
                        bcp[64:128, :], ones_sb[:], recb_p[0:1, G : 2 * G],
                        tile_position=(0, 64),
                    )
                    nc.vector.tensor_tensor(
                        at_p[0:64, :], pvs_p[0:64, 0:G], bcp[0:64, :], op=mult
                    )
                    nc.vector.tensor_tensor(
                        at_p[64:128, :], pvs_p[0:64, G : 2 * G], bcp[64:128, :],
                        op=mult,
                    )

                # ---- evacuate pv psum; queue this body's finish work ----
                pvs = norm_pool.tile([65, 2 * G], f32, tag="pvs")
                nc.vector.tensor_copy(pvs[:, 0:G], pv[0][:])
                nc.vector.tensor_copy(pvs[:, G : 2 * G], pv[1][:])

                # ---- out-projection for the previous body ----
                if pending is not None:
                    _, at_p, _, t0_p = pending
                    outproj(nc, ps_mm, osb_pool, out_d, wo_sb, at_p, t0_p)

                recb = norm_pool.tile([1, 2 * G], bf16, tag="recb")
                at = attn_pool.tile([128, G], bf16, tag="attnT")
                pending = (pvs, at, recb, t0)

            # ---- drain the last body's finish chain ----
            pvs_p, at_p, recb_p, t0_p = pending
            dT = ps_mm.tile([128, G], f32, tag="mm", name="dT")
            for c in range(8):
                nc.tensor.matmul(
                    dT[:, c : c + 1],
                    pvs_p[64:65, c * CH : (c + 1) * CH],
                    ones1_sb[64:65, :],
                )
            recT = norm_pool.tile([128, 8], f32, tag="recT")
            nc.vector.reciprocal(recT[:], dT[:, 0:8])
            rtp = ps_mm.tile([128, G], f32, tag="mm", name="rtp")
            nc.tensor.transpose(rtp[0:8, 0:128], recT[:], ident_sb[:])
            with nc.allow_low_precision(reason="softmax denom recip bf16"):
                for r in range(8):
                    nc.vector.tensor_copy(
                        recb_p[0:1, r * CH : (r + 1) * CH], rtp[r : r + 1, 0:128]
                    )
            bcp = ps_mm.tile([128, G], f32, tag="mm", name="bcp")
            nc.tensor.matmul(bcp[0:64, :], ones_sb[:], recb_p[0:1, 0:G])
            nc.tensor.matmul(
                bcp[64:128, :], ones_sb[:], recb_p[0:1, G : 2 * G],
                tile_position=(0, 64),
            )
            nc.vector.tensor_tensor(
                at_p[0:64, :], pvs_p[0:64, 0:G], bcp[0:64, :], op=mult
            )
            nc.vector.tensor_tensor(
                at_p[64:128, :], pvs_p[0:64, G : 2 * G], bcp[64:128, :], op=mult
            )
            outproj(nc, ps_mm, osb_pool, out_d, wo_sb, at_p, t0_p)
    nc.compile()
    return nc


def outproj(nc, ps_mm, osb_pool, out_d, wo_sb, at, t0):
    import concourse.mybir as mybir

    f32 = mybir.dt.float32
    for tch in range(G // CH):
        ob = osb_pool.tile([128, HID], f32, tag="ob")
        for nn in range(2):
            pso = ps_mm.tile([128, G], f32, tag="mm")
            nc.tensor.matmul(
                pso[:],
                at[:, tch * CH : (tch + 1) * CH],
                wo_sb[:, nn * G : (nn + 1) * G],
            )
            nc.vector.tensor_copy(ob[:, nn * G : (nn + 1) * G], pso[:])
        nc.sync.dma_start(
            out_d[t0 + tch * CH : t0 + (tch + 1) * CH, :],
            ob[:],
        )


def _causal_mask():
    # Packed per-region triangular mask. Diagonal-band chunk jg keeps queries
    # qq >= k within its W = 512 - 128*jg window; layout per jg is
    # [tri(W) | zeros(qoff) | tri(W)] covering [qoff:1024] of the h0|h1 tile.
    tri = (np.arange(G)[None, :] >= np.arange(128)[:, None]).astype(np.float32)
    parts = []
    for jg in range(4):
        qoff = jg * CH
        w = G - qoff
        parts.append(tri[:, 0:w])
        if qoff:
            parts.append(np.zeros((128, qoff), dtype=np.float32))
        parts.append(tri[:, 0:w])
    return np.concatenate(parts, axis=1).astype(ml_dtypes.bfloat16)


def get_nc(reps=1):
    key = f"nc{reps}"
    if key not in _CACHE:
        _CACHE[key] = _build_nc(reps)
    return _CACHE[key]


def make_in_maps(x, w_qkv, w_out):
    bf16 = ml_dtypes.bfloat16
    xf = np.asarray(x, dtype=np.float32).reshape(TT, HID)
    xT = np.ascontiguousarray(xf.T).astype(bf16)
    wqkv = np.asarray(w_qkv, dtype=np.float32)
    wout = np.asarray(w_out, dtype=np.float32)
    mask = _causal_mask()
    in_maps = []
    for c in range(8):
        c0 = 128 * c
        in_maps.append(
            {
                "xT": xT,
                "wq": np.ascontiguousarray(wqkv[:, c0 : c0 + 128]).astype(bf16),
                "wk": np.ascontiguousarray(wqkv[:, HID + c0 : HID + c0 + 128]).astype(bf16),
                "wv": np.ascontiguousarray(wqkv[:, 2 * HID + c0 : 2 * HID + c0 + 128]).astype(bf16),
                "wo": np.ascontiguousarray(wout[c0 : c0 + 128, :]).astype(bf16),
                "mask": mask,
                "ident": np.eye(128, dtype=np.float32),
            }
        )
    return in_maps


def kernel(x, w_qkv, w_out):
    from concourse.bass_utils import run_bass_kernel_spmd

    nc = get_nc()
    in_maps = make_in_maps(x, w_qkv, w_out)
    res = run_bass_kernel_spmd(nc, in_maps, core_ids=list(range(8)))
    acc = np.zeros((TT, HID), dtype=np.float32)
    for r in res.results:
        acc += r["out"]
    return acc.reshape(B, S, HID)
